# revision 1
# baseline (speedup 1.0000x reference)
"""Causal self-attention on 8 trn2 NeuronCores.

Sharding: core c -> (batch b = c // 4, head-group g = c % 4). Each core
computes 4 of the 16 heads for one batch element and the corresponding
slice of the output projection; the host sums the 4 partial projections
per batch and adds the constant bias terms (bv @ Wp.T + bp) exactly.

All transposes (x.T and the weight slices) are done on the host so the
device only runs natural-layout matmuls. Matmuls run as float32r
(full-rate fp32 PE mode); softmax runs unnormalized with the causal mask
applied additively in PSUM, and the 1/rowsum normalization is folded
into the PSUM eviction of the attention output.
"""

import numpy as np

import concourse.bass as bass
import concourse.mybir as mybir
import concourse.tile as tile
from concourse.bass_utils import run_bass_kernel_spmd

B = 2
T = 2048
C = 1024
H = 16
DH = 64
NCORES = 8
GROUPS = 4           # head groups (tensor parallel)
HPG = H // GROUPS    # heads per group = 4
DG = HPG * DH        # head-group width = 256
CHUNK = 512          # query-block size
NCHUNK = T // CHUNK  # 4
KTILE = 128          # key-block size (PE contraction tile)
F32 = mybir.dt.float32
F32R = mybir.dt.float32r
BF16 = mybir.dt.bfloat16
MASK_NEG = -1e30


def _patch_tile_drain():
    """This walrus build lowers Drain/NOP to a CTRL with a single sync-wait
    slot; TileContext's kernel-tail drain accumulates one wait per live
    semaphore and fails codegen. Split the waits across single-wait NOPs."""
    import bass_rust
    from concourse.tile import TileContext

    def _drain_and_barrier_split(self, tick_clock, wait_clock):
        probe = self.nc.sync.nop()
        wait_clock.add_sem_waits(
            probe.ins, tile.ScopedClock({None: tick_clock.global_clock})
        )
        waits = list(probe.ins.sync_info.on_wait or [])
        probe.ins.sync_info.on_wait = []
        # distribute the final-value waits across engines; the all-engine
        # barrier below joins them before the semaphore reset
        engines = [self.nc.sync, self.nc.tensor, self.nc.vector,
                   self.nc.scalar, self.nc.gpsimd]
        for i, w in enumerate(waits):
            n = engines[i % len(engines)].nop()
            if n.ins.sync_info is None:
                n.ins.sync_info = bass_rust.SyncInfo(on_wait=[w], on_update=[])
            else:
                n.ins.sync_info.on_wait = [w]
        self.nc.sync.drain()
        self.nc.all_engine_barrier()
        assert self.sems is not None
        popped = self.nc._tile_sem_poison_stack.pop()
        assert popped is self._sem_poison
        self.nc.clear_and_free_semaphores(list(self.sems.allocated().values()))
        self.nc.all_engine_barrier()

    TileContext._drain_and_barrier = _drain_and_barrier_split

    # Same single-wait limit applies to every lowered TPB instruction (the
    # 64B formats carry one EVENTS field). Post-process the BIR JSON before
    # walrus: hoist extra semaphore waits onto same-engine NoOps.
    import json as _json

    import concourse.bass2jax as bass2jax
    import concourse.bass_utils as bass_utils

    if getattr(bass_utils.compile_bir_kernel, "_wait_split", False):
        return

    _orig_compile = bass_utils.compile_bir_kernel

    def _split_multi_waits(bir_json):
        m = _json.loads(bir_json)
        counter = 0
        changed = False
        for fn in m["functions"]:
            for blk in fn["blocks"]:
                new_insts = []
                for inst in blk["instructions"]:
                    si = inst.get("sync_info")
                    waits = (si or {}).get("on_wait") or []
                    sem_waits = [w for w in waits if w.get("sync_type") == "semaphore"]
                    if len(waits) > 1 and len(sem_waits) == len(waits):
                        changed = True
                        for w in waits[:-1]:
                            counter += 1
                            new_insts.append({
                                "name": f"I-wsplit{counter}",
                                "opcode": "NoOp",
                                "engine": inst["engine"],
                                "ins": [],
                                "outs": [],
                                "sync_info": {"on_wait": [w], "on_update": []},
                            })
                        si["on_wait"] = [waits[-1]]
                    new_insts.append(inst)
                blk["instructions"] = new_insts
        if not changed:
            return bir_json
        return _json.dumps(m).encode()

    def _compile_bir_kernel_split(bir_json, tmpdir, neff_name="file.neff"):
        return _orig_compile(_split_multi_waits(bir_json), tmpdir, neff_name=neff_name)

    _compile_bir_kernel_split._wait_split = True
    bass_utils.compile_bir_kernel = _compile_bir_kernel_split
    bass2jax.compile_bir_kernel = _compile_bir_kernel_split


def build_kernel():
    _patch_tile_drain()
    nc = bass.Bass(target_bir_lowering=False, trn_type="TRN2")

    xT = nc.dram_tensor("xT", [C, T], F32R, kind="ExternalInput")
    wq = nc.dram_tensor("wq", [C, DG], F32R, kind="ExternalInput")
    wk = nc.dram_tensor("wk", [C, DG], F32R, kind="ExternalInput")
    wv = nc.dram_tensor("wv", [C, DG], F32R, kind="ExternalInput")
    wp = nc.dram_tensor("wp", [DG, C], F32R, kind="ExternalInput")
    bq = nc.dram_tensor("bq", [DG], F32, kind="ExternalInput")
    bk = nc.dram_tensor("bk", [DG], F32, kind="ExternalInput")
    out = nc.dram_tensor("out", [T, C], F32, kind="ExternalOutput")

    KO = C // 128            # 8 contraction subtiles for the projections
    MT = DG // 128           # 2 partition tiles for qT/kT and wp rows
    scale = 1.0 / np.sqrt(DH)

    from contextlib import ExitStack

    with tile.TileContext(nc) as tc, ExitStack() as ctx:
        from concourse.masks import make_identity

        const = ctx.enter_context(tc.tile_pool(name="const", bufs=1))
        xt_pool = ctx.enter_context(tc.tile_pool(name="xt", bufs=2))
        persist = ctx.enter_context(tc.tile_pool(name="persist", bufs=1))
        expst_pool = ctx.enter_context(tc.tile_pool(name="expst", bufs=4))
        small = ctx.enter_context(tc.tile_pool(name="small", bufs=4))
        out_pool = ctx.enter_context(tc.tile_pool(name="outp", bufs=3))
        ps_mm = ctx.enter_context(tc.tile_pool(name="psmm", bufs=2, space="PSUM"))
        ps_s = ctx.enter_context(tc.tile_pool(name="pss", bufs=2, space="PSUM"))
        ps_y = ctx.enter_context(tc.tile_pool(name="psy", bufs=2, space="PSUM"))
        dram_pool = ctx.enter_context(tc.tile_pool(name="dram", bufs=2, space="DRAM"))
        onebuf = ctx.enter_context(tc.tile_pool(name="onebuf", bufs=1))

        # ---- constants ----
        xT_r = xT.rearrange("(ko p) t -> p ko t", p=128)
        out_r = out.rearrange("(tt p) c -> tt p c", p=128)
        _xt_tiles = {}

        def prefetch_xt(n):
            if n not in _xt_tiles:
                xt = xt_pool.tile([128, KO, CHUNK], F32R, tag="xt", name=f"xt{n}")
                for ko in range(KO):
                    nc.sync.dma_start(
                        xt[:, ko, :], xT_r[:, ko, n * CHUNK:(n + 1) * CHUNK]
                    )
                _xt_tiles[n] = xt

        def load_xt(n):
            prefetch_xt(n)
            return _xt_tiles.pop(n)

        wq_sb = const.tile([128, KO, DG], F32R)
        nc.sync.dma_start(wq_sb[:], wq.rearrange("(ko p) d -> p ko d", p=128))
        wk_sb = const.tile([128, KO, DG], F32R)
        nc.sync.dma_start(wk_sb[:], wk.rearrange("(ko p) d -> p ko d", p=128))
        prefetch_xt(0)
        wv_sb = const.tile([128, KO, DG], F32R)
        nc.sync.dma_start(wv_sb[:], wv.rearrange("(ko p) d -> p ko d", p=128))
        bq_sb = const.tile([128, MT], F32)
        nc.sync.dma_start(bq_sb[:], bq.rearrange("(mt p) -> p mt", p=128))
        bk_sb = const.tile([128, MT], F32)
        nc.sync.dma_start(bk_sb[:], bk.rearrange("(mt p) -> p mt", p=128))

        ones_f32 = const.tile([128, 64], F32)
        nc.vector.memset(ones_f32[:], 1.0)
        ones_sb = const.tile([128, 64], F32R)
        nc.vector.tensor_copy(ones_sb[:], ones_f32[:])
        ident = const.tile([128, 128], BF16)
        make_identity(nc, ident)
        # wmask[k, j] = MASK_NEG where j < k + 256, else 0.
        # wmask[:, 256:384] is a strict lower-triangle mask; wmask[:, 128:384]
        # additionally blankets 128 fully-masked columns (used on the last
        # diagonal block so its matmuls can run at N=256 instead of N=128).
        wmask = const.tile([128, 384], BF16)
        nc.gpsimd.memset(wmask[:], 0.0)
        nc.gpsimd.affine_select(
            out=wmask[:],
            in_=wmask[:],
            compare_op=mybir.AluOpType.is_ge,
            fill=MASK_NEG,
            base=-256,
            pattern=[[1, 384]],
            channel_multiplier=-1,
        )

        # ---- persistent activations ----
        qT_sb = persist.tile([128, MT, T], F32R)     # [d_local, T] for 4 heads
        kT_sb = persist.tile([128, MT, T], F32R)
        # [tk_in, tk_tile, h, dh+1]; the last column of each head is a ones
        # column so attn@v also accumulates the softmax denominator l.
        v_sb = persist.tile([128, T // 128, HPG, DH + 1], F32R)
        nc.vector.tensor_copy(
            v_sb[:, :, :, DH].rearrange("p t h -> p (t h)"), ones_sb[:]
        )

        def proj(n):
            cols = slice(n * CHUNK, (n + 1) * CHUNK)
            xt = load_xt(n)

            for w_sb, b_sb, dst in ((wq_sb, bq_sb, qT_sb), (wk_sb, bk_sb, kT_sb)):
                for mt in range(MT):
                    ps = ps_mm.tile([128, CHUNK], F32, tag="mm", name=f"pj{n}_{mt}")
                    for ko in range(KO):
                        nc.tensor.matmul(
                            ps[:],
                            lhsT=w_sb[:, ko, mt * 128:(mt + 1) * 128],
                            rhs=xt[:, ko, :],
                            start=(ko == 0),
                            stop=(ko == KO - 1),
                        )
                    nc.vector.tensor_scalar_add(
                        dst[:, mt, cols], ps[:], b_sb[:, mt:mt + 1]
                    )

            for tt in range(CHUNK // 128):
                t_tile = n * (CHUNK // 128) + tt
                ps = ps_mm.tile([128, DG], F32, tag="mm", name=f"pv{n}_{tt}")
                for ko in range(KO):
                    nc.tensor.matmul(
                        ps[:],
                        lhsT=xt[:, ko, tt * 128:(tt + 1) * 128],
                        rhs=wv_sb[:, ko, :],
                        start=(ko == 0),
                        stop=(ko == KO - 1),
                    )
                nc.vector.tensor_copy(v_sb[:, t_tile, :, :DH], ps[:])

        def attention(n, mid=None):
            n_m = 4 * (n + 1)          # causal: key tiles 0 .. 4n+3
            # raw (unnormalized) yT and the denominators, evicted promptly
            # from PSUM so the next pair/chunk can reuse the banks
            yT_raw = small.tile([128, 2, CHUNK], F32, tag="ytr", name=f"ytr{n}")
            l_sb = small.tile([128, CHUNK], F32, tag="lsb", name=f"lsb{n}")

            for p in range(2):
                psy = [
                    ps_y.tile([128, CHUNK], F32, tag="y", name=f"psy{n}_{p}_{i}")
                    for i in range(2)
                ]
                for m in range(n_m):
                    qlo = max(0, 128 * m - CHUNK * n)      # first live column
                    lo = min(qlo, CHUNK - 256)             # keep matmul N >= 256
                    diag = m >= 4 * n

                    pss_t = ps_s.tile(
                        [128, 2, CHUNK], F32, tag="s", name=f"s{n}_{m}_{p}"
                    )
                    for half in range(2):
                        rows = slice(64 * half, 64 * half + 64)
                        nc.tensor.matmul(
                            pss_t[:, half, lo:],
                            lhsT=kT_sb[rows, p, m * 128:(m + 1) * 128],
                            rhs=qT_sb[rows, p, n * CHUNK + lo:(n + 1) * CHUNK],
                            start=True,
                            stop=not diag,
                        )
                    if diag:
                        for half in range(2):
                            if qlo > lo:
                                nc.tensor.matmul(
                                    pss_t[:, half, lo:],
                                    lhsT=ident[:],
                                    rhs=wmask[:, 384 - (CHUNK - lo):],
                                    start=False,
                                    stop=True,
                                )
                            else:
                                nc.tensor.matmul(
                                    pss_t[:, half, qlo:qlo + 128],
                                    lhsT=ident[:],
                                    rhs=wmask[:, 256:384],
                                    start=False,
                                    stop=True,
                                )

                    e = expst_pool.tile([128, 2, CHUNK], F32R, tag="e")
                    nc.scalar.activation(
                        e[:, :, lo:], pss_t[:, :, lo:],
                        mybir.ActivationFunctionType.Exp, scale=scale,
                    )

                    for half in range(2):
                        h = 2 * p + half
                        nc.tensor.matmul(
                            psy[half][0:DH + 1, lo:],
                            lhsT=v_sb[:, m, h, :],
                            rhs=e[:, half, lo:],
                            start=(m == 0),
                            stop=(m == n_m - 1),
                        )

                for half in range(2):
                    h = 2 * p + half
                    nc.vector.tensor_copy(
                        yT_raw[64 * half:64 * half + 64, p, :], psy[half][0:DH, :]
                    )
                    nc.vector.tensor_copy(
                        l_sb[32 * h:32 * h + 1, :], psy[half][DH:DH + 1, :]
                    )
                if p == 0 and mid is not None:
                    mid(yT_raw, l_sb)
            return yT_raw, l_sb

        def normalize(n, yT_raw, l_sb):
            recip = small.tile([128, CHUNK], F32, tag="recip", name=f"rc{n}")
            for h in range(HPG):
                nc.vector.reciprocal(
                    recip[32 * h:32 * h + 1, :], l_sb[32 * h:32 * h + 1, :]
                )
            # partition-broadcast of recip rows via a DRAM round-trip (the
            # only 0-step-partition DMA this toolchain accepts)
            recip_dr = dram_pool.tile([HPG, CHUNK], F32, tag="rdr")
            for h in range(HPG):
                nc.sync.dma_start(
                    recip_dr[h:h + 1, :], recip[32 * h:32 * h + 1, :]
                )
            yT_n = small.tile([128, 2, CHUNK], F32R, tag="yt", name=f"yn{n}")
            for ks in range(2):
                bc = small.tile([128, CHUNK], F32, tag="bc", name=f"bc{n}_{ks}")
                for half in range(2):
                    h = 2 * ks + half
                    nc.sync.dma_start(
                        bc[64 * half:64 * half + 64, :],
                        recip_dr[h:h + 1, :].to_broadcast((64, CHUNK)),
                    )
                for half in range(2):
                    nc.vector.tensor_mul(
                        yT_n[64 * half:64 * half + 64, ks, :],
                        yT_raw[64 * half:64 * half + 64, ks, :],
                        bc[64 * half:64 * half + 64, :],
                    )
            return yT_n

        def normalize_fast_ks(n, yT_raw, l_sb, yT_n, rfast, ks):
            """Epilogue variant: broadcast 1/l via a K=1 PE outer-product into
            PSUM instead of the DRAM round-trip, to shorten the tail. One ks
            (head pair) at a time so pair 0 can run mid-attention."""
            with nc.allow_low_precision(reason="f32r is 4-byte fp32 storage"):
                for half in range(2):
                    h = 2 * ks + half
                    nc.vector.reciprocal(
                        rfast[32 * h:32 * h + 1, :], l_sb[32 * h:32 * h + 1, :]
                    )
            bc_ps = [
                ps_mm.tile([128, 512], F32, tag="mm", name=f"bcp{n}_{ks}_{i}")
                for i in range(2)
            ]
            for half in range(2):
                h = 2 * ks + half
                nc.tensor.matmul(
                    bc_ps[half][0:64, :],
                    lhsT=ones_sb[32 * h:32 * h + 1, :],
                    rhs=rfast[32 * h:32 * h + 1, :],
                    start=True,
                    stop=True,
                    tile_position=(32 * h, 0),
                )
            for half in range(2):
                nc.vector.tensor_mul(
                    yT_n[64 * half:64 * half + 64, ks, :],
                    yT_raw[64 * half:64 * half + 64, ks, :],
                    bc_ps[half][0:64, :],
                )

        def outproj(n, yT_n):
            for tt in range(CHUNK // 128):
                t_tile = n * (CHUNK // 128) + tt
                o_sb = out_pool.tile([128, C], F32, tag="o", name=f"o{n}_{tt}")
                for nh in range(2):
                    ps = ps_mm.tile([128, 512], F32, tag="mm", name=f"po{n}_{tt}_{nh}")
                    for ks in range(MT):
                        nc.tensor.matmul(
                            ps[:],
                            lhsT=yT_n[:, ks, tt * 128:(tt + 1) * 128],
                            rhs=wp_box[0][:, ks, nh * 512:(nh + 1) * 512],
                            start=(ks == 0),
                            stop=(ks == MT - 1),
                        )
                    nc.any.tensor_copy(o_sb[:, nh * 512:(nh + 1) * 512], ps[:])
                nc.sync.dma_start(out_r[t_tile], o_sb[:])

        # software pipeline: normalize+outproj for chunk n-1 are emitted
        # after attention(n) so the PE stream never stalls on the
        # normalization round-trip latency
        wp_box = []

        def load_wp():
            wp_sb = const.tile([128, MT, C], F32R)
            nc.sync.dma_start(wp_sb[:], wp.rearrange("(mt p) c -> p mt c", p=128))
            wp_box.append(wp_sb)

        pending = None
        last = NCHUNK - 1
        yn_last = small.tile([128, 2, CHUNK], F32R, tag="yt", name="ynlast")
        rf_last = onebuf.tile([128, CHUNK], F32R, tag="rfast", name="rflast")
        for n in range(NCHUNK):
            proj(n)
            if n == 0:
                load_wp()
            normed = []
            if pending is not None:
                pn, (yr, ls) = pending

                def mid(cur_yr, cur_ls, pn=pn, yr=yr, ls=ls, is_last=(n == last)):
                    normed.append(normalize(pn, yr, ls))
                    if is_last:
                        # pair 0 of the final chunk is done: normalize its
                        # half now so only pair 1's half trails the loop
                        normalize_fast_ks(last, cur_yr, cur_ls, yn_last,
                                          rf_last, 0)
            else:
                mid = None
            state = attention(n, mid=mid)
            if pending is not None:
                outproj(pn, normed[0])
            pending = (n, state)
        pn, (yr, ls) = pending
        normalize_fast_ks(pn, yr, ls, yn_last, rf_last, 1)
        outproj(pn, yn_last)

    return nc


_NC_CACHE = None


def kernel(**inputs) -> np.ndarray:
    global _NC_CACHE
    x = np.asarray(inputs["x"], np.float32)
    Wq = np.asarray(inputs["Wq"], np.float32)
    Wk = np.asarray(inputs["Wk"], np.float32)
    Wv = np.asarray(inputs["Wv"], np.float32)
    Wp = np.asarray(inputs["Wp"], np.float32)
    bq = np.asarray(inputs["bq"], np.float32)
    bk = np.asarray(inputs["bk"], np.float32)
    bv = np.asarray(inputs["bv"], np.float32)
    bp = np.asarray(inputs["bp"], np.float32)

    if _NC_CACHE is None:
        _NC_CACHE = build_kernel()
    nc = _NC_CACHE

    in_maps = []
    for c in range(NCORES):
        b, g = divmod(c, GROUPS)
        rows = slice(g * DG, (g + 1) * DG)
        in_maps.append({
            "xT": np.ascontiguousarray(x[b].T),
            "wq": np.ascontiguousarray(Wq[rows, :].T),
            "wk": np.ascontiguousarray(Wk[rows, :].T),
            "wv": np.ascontiguousarray(Wv[rows, :].T),
            "wp": np.ascontiguousarray(Wp[:, rows].T),
            "bq": np.ascontiguousarray(bq[rows]),
            "bk": np.ascontiguousarray(bk[rows]),
        })

    res = run_bass_kernel_spmd(nc, in_maps, core_ids=list(range(NCORES)))

    result = np.zeros((B, T, C), np.float32)
    for c in range(NCORES):
        b = c // GROUPS
        result[b] += res.results[c]["out"]
    result += (bv @ Wp.T + bp)[None, None, :]
    return result



# revision 27
# speedup vs baseline: 1.0952x; 1.0952x over previous
"""Causal self-attention on 8 trn2 NeuronCores.

Sharding: core c -> (batch b = c // 4, head-group g = c % 4). Each core
computes 4 of the 16 heads for one batch element plus its slice of the
output projection; the host sums the 4 partial projections per batch and
adds the constant (bv @ Wp.T + bp) term exactly.

Kernel structure (per core), streamed over 4 query chunks of 512:
  - Q/K/V projections as fp8e4 DoubleRow matmuls with hi+lo residual
    splits of both x and W (3 accumulation terms; quantization error
    ~0.05%), contraction 256/step.
  - Scores s = k.T q in fp8e4 DoubleRow ([keys, queries] orientation,
    dh packed 32x2), causal mask added in PSUM via an identity matmul,
    exp on the Activation engine straight out of PSUM into bf16 SBUF.
  - attn@v flipped: e is the stationary operand, v (with a trailing
    ones column that accumulates the softmax denominator l) is moving;
    PSUM rows are queries so 1/l is a per-partition scalar folded into
    the eviction tensor_scalar op.
  - y transposed via the DMA xbar (16x128 tiles) into [ydim, t] layout,
    then the output projection in bf16; out partials stored bf16.
"""

import numpy as np
import ml_dtypes

import concourse.bass as bass
import concourse.mybir as mybir
import concourse.tile as tile
from concourse.bass_utils import run_bass_kernel_spmd

B = 2
T = 2048
C = 1024
H = 16
DH = 64
NCORES = 8
GROUPS = 4            # head groups (tensor parallel)
HPG = H // GROUPS     # heads per group = 4
DG = HPG * DH         # head-group width = 256
CHUNK = 512           # query-chunk size
NCHUNK = T // CHUNK   # 4
KO2 = C // 256        # DoubleRow contraction steps for the projections
NKT = T // 128        # key tiles
F32 = mybir.dt.float32
F32R = mybir.dt.float32r
BF16 = mybir.dt.bfloat16
F8 = mybir.dt.float8e4
NPF8 = ml_dtypes.float8_e4m3
NPBF16 = ml_dtypes.bfloat16
MASK_NEG = -1e30

SX = 16.0             # x fp8 scale
SW = 256.0            # weight fp8 scale
SQ = 4.0              # q/k fp8 store scale
DR = mybir.MatmulPerfMode.DoubleRow


def _patch_tile_drain():
    """This walrus build lowers Drain/NOP to a CTRL with a single sync-wait
    slot; TileContext's kernel-tail drain accumulates one wait per live
    semaphore and fails codegen. Split the waits across single-wait NOPs."""
    import bass_rust
    from concourse.tile import TileContext

    def _drain_and_barrier_split(self, tick_clock, wait_clock):
        probe = self.nc.sync.nop()
        wait_clock.add_sem_waits(
            probe.ins, tile.ScopedClock({None: tick_clock.global_clock})
        )
        waits = list(probe.ins.sync_info.on_wait or [])
        probe.ins.sync_info.on_wait = []
        engines = [self.nc.sync, self.nc.tensor, self.nc.vector,
                   self.nc.scalar, self.nc.gpsimd]
        for i, w in enumerate(waits):
            n = engines[i % len(engines)].nop()
            if n.ins.sync_info is None:
                n.ins.sync_info = bass_rust.SyncInfo(on_wait=[w], on_update=[])
            else:
                n.ins.sync_info.on_wait = [w]
        self.nc.sync.drain()
        self.nc.all_engine_barrier()
        assert self.sems is not None
        popped = self.nc._tile_sem_poison_stack.pop()
        assert popped is self._sem_poison
        self.nc.clear_and_free_semaphores(list(self.sems.allocated().values()))
        self.nc.all_engine_barrier()

    TileContext._drain_and_barrier = _drain_and_barrier_split

    import json as _json

    import concourse.bass2jax as bass2jax
    import concourse.bass_utils as bass_utils

    if getattr(bass_utils.compile_bir_kernel, "_wait_split", False):
        return

    _orig_compile = bass_utils.compile_bir_kernel

    def _split_multi_waits(bir_json):
        m = _json.loads(bir_json)
        counter = 0
        changed = False
        for fn in m["functions"]:
            for blk in fn["blocks"]:
                new_insts = []
                for inst in blk["instructions"]:
                    si = inst.get("sync_info")
                    waits = (si or {}).get("on_wait") or []
                    sem_waits = [w for w in waits if w.get("sync_type") == "semaphore"]
                    if len(waits) > 1 and len(sem_waits) == len(waits):
                        changed = True
                        for w in waits[:-1]:
                            counter += 1
                            new_insts.append({
                                "name": f"I-wsplit{counter}",
                                "opcode": "NoOp",
                                "engine": inst["engine"],
                                "ins": [],
                                "outs": [],
                                "sync_info": {"on_wait": [w], "on_update": []},
                            })
                        si["on_wait"] = [waits[-1]]
                    new_insts.append(inst)
                blk["instructions"] = new_insts
        if not changed:
            return bir_json
        return _json.dumps(m).encode()

    def _compile_bir_kernel_split(bir_json, tmpdir, neff_name="file.neff"):
        return _orig_compile(_split_multi_waits(bir_json), tmpdir, neff_name=neff_name)

    _compile_bir_kernel_split._wait_split = True
    bass_utils.compile_bir_kernel = _compile_bir_kernel_split
    bass2jax.compile_bir_kernel = _compile_bir_kernel_split


def build_kernel():
    _patch_tile_drain()
    nc = bass.Bass(target_bir_lowering=False, trn_type="TRN2")

    # hi/lo fp8 operand pairs; layouts are DoubleRow-packed on the host:
    # contraction index c = ko*256 + slot*128 + p.
    xh = nc.dram_tensor("xh", [NCHUNK, 128, 2, KO2, CHUNK], F8, kind="ExternalInput")
    xl = nc.dram_tensor("xl", [NCHUNK, 128, 2, KO2, CHUNK], F8, kind="ExternalInput")
    wqh = nc.dram_tensor("wqh", [128, 2, KO2, DG], F8, kind="ExternalInput")
    wql = nc.dram_tensor("wql", [128, 2, KO2, DG], F8, kind="ExternalInput")
    wkh = nc.dram_tensor("wkh", [128, 2, KO2, DG], F8, kind="ExternalInput")
    wkl = nc.dram_tensor("wkl", [128, 2, KO2, DG], F8, kind="ExternalInput")
    wvh = nc.dram_tensor("wvh", [128, 2, KO2, DG], F8, kind="ExternalInput")
    wvl = nc.dram_tensor("wvl", [128, 2, KO2, DG], F8, kind="ExternalInput")
    wpt = nc.dram_tensor("wpt", [128, 2, C], BF16, kind="ExternalInput")
    bq4 = nc.dram_tensor("bq4", [128, 2], F32, kind="ExternalInput")
    bk4 = nc.dram_tensor("bk4", [128, 2], F32, kind="ExternalInput")
    out = nc.dram_tensor("out", [NKT, 128, C], BF16, kind="ExternalOutput")

    from contextlib import ExitStack

    with tile.TileContext(nc) as tc, ExitStack() as ctx:
        from concourse.masks import make_identity

        const = ctx.enter_context(tc.tile_pool(name="const", bufs=1))
        xpool = ctx.enter_context(tc.tile_pool(name="xp", bufs=4))
        persist = ctx.enter_context(tc.tile_pool(name="persist", bufs=1))
        epool = ctx.enter_context(tc.tile_pool(name="ep", bufs=4))
        ypool = ctx.enter_context(tc.tile_pool(name="yp", bufs=2))
        ytpool = ctx.enter_context(tc.tile_pool(name="ytp", bufs=4))
        opool = ctx.enter_context(tc.tile_pool(name="op", bufs=3))
        small = ctx.enter_context(tc.tile_pool(name="sm", bufs=6))
        ps_big = ctx.enter_context(tc.tile_pool(name="psb", bufs=2, space="PSUM"))
        ps_y = ctx.enter_context(tc.tile_pool(name="psy", bufs=2, space="PSUM"))
        ps_o = ctx.enter_context(tc.tile_pool(name="pso", bufs=2, space="PSUM"))

        _x_tiles = {}

        def prefetch_x(n):
            if n not in _x_tiles and n < NCHUNK:
                th = xpool.tile([128, 2, KO2, CHUNK], F8, tag="x", name=f"xh{n}")
                nc.sync.dma_start(th[:], xh[n])
                tl = xpool.tile([128, 2, KO2, CHUNK], F8, tag="x", name=f"xl{n}")
                nc.sync.dma_start(tl[:], xl[n])
                _x_tiles[n] = (th, tl)

        def load_x(n):
            prefetch_x(n)
            return _x_tiles.pop(n)

        # ---- constants ----  (x chunk 0 is prefetched right after the wq
        # pair so the first projection matmul can start ~2.5us in)
        wq_sb, wk_sb, wv_sb = [], [], []
        _w_srcs = ((wq_sb, wqh, wql), (wk_sb, wkh, wkl), (wv_sb, wvh, wvl))
        _w_tiles = []
        for wn, (dst, hi, lo) in enumerate(_w_srcs):
            for hl, w_dram in enumerate((hi, lo)):
                t = const.tile([128, 2, KO2, DG], F8, name=f"w{wn}_{hl}")
                _w_tiles.append((t, w_dram))
                dst.append(t)
        _order = [0, 2, 1, 3, 4, 5]        # wq-hi, wk-hi, wq-lo, wk-lo, wv
        nc.sync.dma_start(_w_tiles[0][0][:], _w_tiles[0][1][:])   # wq hi
        nc.sync.dma_start(_w_tiles[2][0][:], _w_tiles[2][1][:])   # wk hi
        prefetch_x(0)
        for wi in (1, 3, 4, 5):
            t, w_dram = _w_tiles[wi]
            nc.sync.dma_start(t[:], w_dram[:])
        bq_sb = const.tile([128, 2], F32)
        nc.sync.dma_start(bq_sb[:], bq4[:])
        bk_sb = const.tile([128, 2], F32)
        nc.sync.dma_start(bk_sb[:], bk4[:])
        wpt_sb = const.tile([128, 2, C], BF16)
        nc.sync.dma_start(wpt_sb[:], wpt[:])

        ident = const.tile([128, 128], BF16)
        make_identity(nc, ident)
        ones_row = const.tile([1, 128], BF16)
        nc.vector.memset(ones_row[:], 1.0)
        zero_row = const.tile([1, 512], BF16)
        nc.vector.memset(zero_row[:], 0.0)
        # wmask[k, q] = 0 where q >= k else MASK_NEG (strict lower triangle
        # of keys over queries within the diagonal 128x128 block)
        wmask = const.tile([128, 128], BF16)
        nc.gpsimd.memset(wmask[:], 0.0)
        nc.gpsimd.affine_select(
            out=wmask[:],
            in_=wmask[:],
            compare_op=mybir.AluOpType.is_ge,
            fill=MASK_NEG,
            base=0,
            pattern=[[1, 128]],
            channel_multiplier=-1,
        )

        # ---- persistent activations ----
        # q8/k8: partition = (h%2)*64 + dh, free dims (pair, drslot, t).
        # drslot 1 is a constant zero operand: DoubleRow needs a [p, 2, n]
        # shape but the contraction is only 64 deep, so the second slot
        # multiplies zeros (and must be zeroed -- fp8 garbage can be NaN).
        q8 = persist.tile([128, 2, 2, T], F8)
        k8 = persist.tile([128, 2, 2, T], F8)
        nc.gpsimd.memset(q8[:, :, 1, :], 0.0)
        nc.gpsimd.memset(k8[:, :, 1, :], 0.0)
        # v: partition = key%128, free (ktile, head, dh+ones)
        v_sb = persist.tile([128, NKT, HPG, DH + 1], BF16)
        nc.vector.memset(v_sb[:, :, :, DH:DH + 1], 1.0)

        def proj_units(n):
            """Chunk-n projection emission as self-contained closures (one
            complete PSUM accumulation group each) so they can be spread
            across the previous chunk's exp-bound attention phase."""
            if n >= NCHUNK:
                return []
            cols = slice(n * CHUNK, (n + 1) * CHUNK)
            xs = {}

            def get_x():
                if "x" not in xs:
                    xs["x"] = load_x(n)
                    prefetch_x(n + 1)
                return xs["x"]

            units = []

            def qk_unit(w_pair, b_sb, dst, mt, tag):
                def run():
                    xthi, xtlo = get_x()
                    terms = ((xthi, 0), (xthi, 1), (xtlo, 0))
                    ps = ps_big.tile([128, 2, CHUNK], F32, tag="ps",
                                     name=f"p{tag}{n}_{mt}")
                    i, nmm = 0, len(terms) * KO2
                    for xt, wi in terms:
                        for ko in range(KO2):
                            nc.tensor.matmul(
                                ps[:, 0, :],
                                lhsT=w_pair[wi][:, :, ko, mt * 128:(mt + 1) * 128],
                                rhs=xt[:, :, ko, :],
                                start=(i == 0),
                                stop=(i == nmm - 1),
                                perf_mode=DR,
                            )
                            i += 1
                    nc.vector.tensor_scalar(
                        dst[:, mt, 0, cols], ps[:, 0, :],
                        SQ / (SX * SW), b_sb[:, mt:mt + 1],
                        op0=mybir.AluOpType.mult, op1=mybir.AluOpType.add,
                    )
                return run

            def v_unit(tt):
                def run():
                    xthi, xtlo = get_x()
                    terms = ((xthi, 0), (xthi, 1), (xtlo, 0))
                    kt = 4 * n + tt
                    ps = ps_big.tile([128, 2, CHUNK], F32, tag="ps",
                                     name=f"pv{n}_{tt}")
                    i, nmm = 0, len(terms) * KO2
                    for xt, wi in terms:
                        for ko in range(KO2):
                            nc.tensor.matmul(
                                ps[:, 0, 0:DG],
                                lhsT=xt[:, :, ko, tt * 128:(tt + 1) * 128],
                                rhs=wv_sb[wi][:, :, ko, :],
                                start=(i == 0),
                                stop=(i == nmm - 1),
                                perf_mode=DR,
                            )
                            i += 1
                    nc.vector.tensor_scalar_mul(
                        v_sb[:, kt, :, 0:DH], ps[:, 0, 0:DG], 1.0 / (SX * SW)
                    )
                return run

            for mt in range(2):
                units.append(qk_unit(wq_sb, bq_sb, q8, mt, "q"))
                units.append(qk_unit(wk_sb, bk_sb, k8, mt, "k"))
            for tt in range(4):
                units.append(v_unit(tt))
            return units

        DELAY = 3

        carry_out = []

        def attention(n, feed):
            """Scores + attn@v for chunk n (two head-pair passes). attn@v
            runs DELAY score-tiles behind the exp producing its input, and
            units from `feed` (next chunk's projection) are spread evenly
            over the score tiles to fill the PE while ACT works through
            the exps."""
            njt = 4 * (n + 1)
            steps_total = 2 * njt
            step_no = [0]
            nfeed = len(feed)
            consumed = [0]
            carried = list(carry_out)
            carry_out.clear()
            y_sb = ypool.tile([128, 4, HPG, DH], BF16, tag="y", name=f"y{n}")
            yt_sb = ytpool.tile([128, 2, CHUNK], BF16, tag="yt", name=f"yt{n}")
            pending_av = []
            pending_out = []

            def normalize(p, qt, psyA, psyB):
                rec = small.tile([128, 2], F32, tag="rec", name=f"rc{n}_{p}_{qt}")
                lv = small.tile([128, 2], F32, tag="lv", name=f"lv{n}_{p}_{qt}")
                nc.vector.tensor_copy(lv[:, 0:1], psyA[:, qt, DH:DH + 1])
                nc.vector.tensor_copy(lv[:, 1:2], psyB[:, qt, DH:DH + 1])
                nc.vector.reciprocal(rec[:], lv[:])
                for h01, psy in ((0, psyA), (1, psyB)):
                    nc.vector.tensor_scalar_mul(
                        y_sb[:, qt, 2 * p + h01, :], psy[:, qt, 0:DH],
                        rec[:, h01:h01 + 1],
                    )

            def transpose_y(qt):
                # y[q, ydim] -> yT[ydim, t] via the PE transpose datapath;
                # output lands bf16 in a bitcast corner of an outproj-pool
                # tile (keeps the scores-pool rotation free of chunk-tail
                # eviction dependencies)
                tp = ps_big.tile([128, 2, CHUNK], F32, tag="ps", name=f"tp{n}_{qt}")
                tpb = tp[:].bitcast(BF16)
                for mt in range(2):
                    nc.tensor.matmul(
                        tpb[:, mt, 0:128],
                        lhsT=y_sb[:, qt, 2 * mt:2 * mt + 2, :],
                        rhs=ident[:],
                        is_transpose=True,
                    )
                nc.vector.tensor_copy(
                    yt_sb[:, :, qt * 128:(qt + 1) * 128], tpb[:, :, 0:128]
                )

            def outproj(qt):
                t_tile = 4 * n + qt
                o_sb = opool.tile([128, C], BF16, tag="o", name=f"o{n}_{qt}")
                for nh in range(2):
                    ps = ps_o.tile([128, 512], F32, tag="o", name=f"po{n}_{qt}_{nh}")
                    for mt in range(2):
                        nc.tensor.matmul(
                            ps[:],
                            lhsT=yt_sb[:, mt, qt * 128:(qt + 1) * 128],
                            rhs=wpt_sb[:, mt, nh * 512:(nh + 1) * 512],
                            start=(mt == 0),
                            stop=(mt == 1),
                        )
                    nc.vector.tensor_copy(o_sb[:, nh * 512:(nh + 1) * 512], ps[:])
                nc.sync.dma_start(out[t_tile], o_sb[:])

            def attnv(p, j, e, psyA, psyB):
                def run():
                    qlo = max(0, 128 * (j - 4 * n))
                    for qt in range(qlo // 128, 4):
                        for h01, psy in ((0, psyA), (1, psyB)):
                            nc.tensor.matmul(
                                psy[:, qt, 0:DH + 1],
                                lhsT=e[:, h01, qt * 128:(qt + 1) * 128],
                                rhs=v_sb[:, j, 2 * p + h01, :],
                                start=False,
                                stop=(j == 4 * n + qt),
                                skip_group_check=True,
                            )
                    if j >= 4 * n:
                        qt_done = j - 4 * n
                        normalize(p, qt_done, psyA, psyB)
                        if p == 1:
                            transpose_y(qt_done)
                            pending_out.append(qt_done)
                            if len(pending_out) > 1:
                                outproj(pending_out.pop(0))
                return run

            def step():
                if pending_av and len(pending_av) > DELAY:
                    pending_av.pop(0)()
                step_no[0] += 1
                if carried and step_no[0] % 2 == 0:
                    carried.pop(0)()
                # spread feed units so they finish ~7/8 through the chunk
                if feed:
                    target = min(len(feed) + consumed[0],
                                 (nfeed * 8 * step_no[0])
                                 // (7 * steps_total) + 1)
                    while consumed[0] < target and feed:
                        feed.pop(0)()
                        consumed[0] += 1

            for p in range(2):
                psyA = ps_y.tile([128, 4, 128], F32, tag="psy", name=f"pyA{n}_{p}")
                psyB = ps_y.tile([128, 4, 128], F32, tag="psy", name=f"pyB{n}_{p}")
                # one start per PSUM bank: a K=1 matmul zeroes the whole
                # bank so every attn@v slot can accumulate with start=False
                # (multiple start groups in one 2KB region clobber siblings)
                for psy in (psyA, psyB):
                    nc.tensor.matmul(
                        psy[:, :, :].rearrange("p a b -> p (a b)"),
                        lhsT=ones_row[:],
                        rhs=zero_row[:],
                        start=True,
                        stop=True,
                        skip_group_check=True,
                    )
                for j in range(njt):
                    qlo = max(0, 128 * (j - 4 * n))
                    diag = j >= 4 * n
                    pss = ps_big.tile([128, 2, CHUNK], F32, tag="ps",
                                      name=f"ss{n}_{p}_{j}")
                    for h01 in range(2):
                        rows = slice(64 * h01, 64 * h01 + 64)
                        nc.tensor.matmul(
                            pss[:, h01, qlo:],
                            lhsT=k8[rows, p, :, j * 128:(j + 1) * 128],
                            rhs=q8[rows, p, :,
                                   n * CHUNK + qlo:(n + 1) * CHUNK],
                            start=True,
                            stop=not diag,
                            perf_mode=DR,
                        )
                    if diag:
                        for h01 in range(2):
                            nc.tensor.matmul(
                                pss[:, h01, qlo:qlo + 128],
                                lhsT=ident[:],
                                rhs=wmask[:],
                                start=False,
                                stop=True,
                            )
                    e = epool.tile([128, 2, CHUNK], BF16, tag="e")
                    nc.scalar.activation(
                        e[:, :, qlo:], pss[:, :, qlo:],
                        mybir.ActivationFunctionType.Exp,
                        scale=1.0 / (8.0 * SQ * SQ),
                    )
                    pending_av.append(attnv(p, j, e, psyA, psyB))
                    step()
            while pending_av:
                pending_av.pop(0)()
                while feed:
                    feed.pop(0)()
            for fn in carried:
                fn()
            if n < NCHUNK - 1:
                for qt in pending_out:
                    carry_out.append(lambda qt=qt: outproj(qt))
            else:
                for qt in pending_out:
                    outproj(qt)

        for u in proj_units(0):
            u()
        for n in range(NCHUNK):
            feed = proj_units(n + 1)
            attention(n, feed)
            for u in feed:
                u()

    return nc


_NC_CACHE = None


def _split8(a, s):
    hi = np.asarray(a * s, NPF8)
    lo = np.asarray(a * s - hi.astype(np.float32), NPF8)
    return hi, lo


def kernel(**inputs) -> np.ndarray:
    global _NC_CACHE
    x = np.asarray(inputs["x"], np.float32)
    Wq = np.asarray(inputs["Wq"], np.float32)
    Wk = np.asarray(inputs["Wk"], np.float32)
    Wv = np.asarray(inputs["Wv"], np.float32)
    Wp = np.asarray(inputs["Wp"], np.float32)
    bq = np.asarray(inputs["bq"], np.float32)
    bk = np.asarray(inputs["bk"], np.float32)
    bv = np.asarray(inputs["bv"], np.float32)
    bp = np.asarray(inputs["bp"], np.float32)

    if _NC_CACHE is None:
        _NC_CACHE = build_kernel()
    nc = _NC_CACHE

    def pack_w(Wl):
        # Wl: [256 out, 1024 in] slice -> lhsT [c, m] -> [p, slot, ko, m]
        wt = Wl.T                                          # [1024 c, 256 m]
        wt = wt.reshape(KO2, 2, 128, 256).transpose(2, 1, 0, 3)
        hi, lo = _split8(np.ascontiguousarray(wt), SW)
        return np.ascontiguousarray(hi), np.ascontiguousarray(lo)

    in_maps = []
    for c in range(NCORES):
        b, g = divmod(c, GROUPS)
        rows = slice(g * DG, (g + 1) * DG)
        xt = x[b].T.reshape(KO2, 2, 128, T).transpose(2, 1, 0, 3)  # [p,slot,ko,t]
        xt = xt.reshape(128, 2, KO2, NCHUNK, CHUNK).transpose(3, 0, 1, 2, 4)
        xhi, xlo = _split8(np.ascontiguousarray(xt), SX)

        wq_hi, wq_lo = pack_w(Wq[rows])
        wk_hi, wk_lo = pack_w(Wk[rows])
        wv_hi, wv_lo = pack_w(Wv[rows])
        wpt_l = np.ascontiguousarray(
            Wp[:, rows].T.reshape(2, 128, C).transpose(1, 0, 2)
        ).astype(NPBF16)

        bq4 = np.ascontiguousarray((bq[rows] * SQ).reshape(2, 128).T)
        bk4 = np.ascontiguousarray((bk[rows] * SQ).reshape(2, 128).T)

        in_maps.append({
            "xh": np.ascontiguousarray(xhi),
            "xl": np.ascontiguousarray(xlo),
            "wqh": wq_hi, "wql": wq_lo,
            "wkh": wk_hi, "wkl": wk_lo,
            "wvh": wv_hi, "wvl": wv_lo,
            "wpt": wpt_l,
            "bq4": bq4, "bk4": bk4,
        })

    res = run_bass_kernel_spmd(nc, in_maps, core_ids=list(range(NCORES)))

    result = np.zeros((B, T, C), np.float32)
    for c in range(NCORES):
        b = c // GROUPS
        o = np.asarray(res.results[c]["out"]).astype(np.float32)
        result[b] += o.reshape(T, C)
    result += (bv @ Wp.T + bp)[None, None, :]
    return result


# revision 37
# speedup vs baseline: 1.1814x; 1.0787x over previous
"""Causal self-attention on 8 trn2 NeuronCores.

Sharding: core c -> (batch b = c // 4, head-group g = c % 4). Each core
computes 4 of the 16 heads for one batch element plus its slice of the
output projection; the host sums the 4 partial projections per batch and
adds the constant (bv @ Wp.T + bp) term exactly.

Kernel structure (per core), streamed over 4 query chunks of 512:
  - Q/K/V projections as fp8e4 DoubleRow matmuls with hi+lo residual
    splits of both x and W (3 accumulation terms; quantization error
    ~0.05%), contraction 256/step.
  - Scores s = k.T q in fp8e4 DoubleRow ([keys, queries] orientation,
    dh packed 32x2), causal mask added in PSUM via an identity matmul,
    exp on the Activation engine straight out of PSUM into bf16 SBUF.
  - attn@v flipped: e is the stationary operand, v (with a trailing
    ones column that accumulates the softmax denominator l) is moving;
    PSUM rows are queries so 1/l is a per-partition scalar folded into
    the eviction tensor_scalar op.
  - y transposed via the DMA xbar (16x128 tiles) into [ydim, t] layout,
    then the output projection in bf16; out partials stored bf16.
"""

import numpy as np
import ml_dtypes

import concourse.bass as bass
import concourse.mybir as mybir
import concourse.tile as tile
from concourse.bass_utils import run_bass_kernel_spmd

B = 2
T = 2048
C = 1024
H = 16
DH = 64
NCORES = 8
GROUPS = 4            # head groups (tensor parallel)
HPG = H // GROUPS     # heads per group = 4
DG = HPG * DH         # head-group width = 256
CHUNK = 512           # query-chunk size
NCHUNK = T // CHUNK   # 4
KO2 = C // 256        # DoubleRow contraction steps for the projections
NKT = T // 128        # key tiles
F32 = mybir.dt.float32
F32R = mybir.dt.float32r
BF16 = mybir.dt.bfloat16
F8 = mybir.dt.float8e4
NPF8 = ml_dtypes.float8_e4m3
NPBF16 = ml_dtypes.bfloat16
MASK_NEG = -1e30

SX = 16.0             # x fp8 scale
SW = 256.0            # weight fp8 scale
SQ = 4.0              # q/k fp8 store scale
DR = mybir.MatmulPerfMode.DoubleRow


def _patch_tile_drain():
    """This walrus build lowers Drain/NOP to a CTRL with a single sync-wait
    slot; TileContext's kernel-tail drain accumulates one wait per live
    semaphore and fails codegen. Split the waits across single-wait NOPs."""
    import bass_rust
    from concourse.tile import TileContext

    def _drain_and_barrier_split(self, tick_clock, wait_clock):
        probe = self.nc.sync.nop()
        wait_clock.add_sem_waits(
            probe.ins, tile.ScopedClock({None: tick_clock.global_clock})
        )
        waits = list(probe.ins.sync_info.on_wait or [])
        probe.ins.sync_info.on_wait = []
        engines = [self.nc.sync, self.nc.tensor, self.nc.vector,
                   self.nc.scalar, self.nc.gpsimd]
        for i, w in enumerate(waits):
            n = engines[i % len(engines)].nop()
            if n.ins.sync_info is None:
                n.ins.sync_info = bass_rust.SyncInfo(on_wait=[w], on_update=[])
            else:
                n.ins.sync_info.on_wait = [w]
        self.nc.sync.drain()
        self.nc.all_engine_barrier()
        assert self.sems is not None
        popped = self.nc._tile_sem_poison_stack.pop()
        assert popped is self._sem_poison
        self.nc.clear_and_free_semaphores(list(self.sems.allocated().values()))
        self.nc.all_engine_barrier()

    TileContext._drain_and_barrier = _drain_and_barrier_split

    import json as _json

    import concourse.bass2jax as bass2jax
    import concourse.bass_utils as bass_utils

    if getattr(bass_utils.compile_bir_kernel, "_wait_split", False):
        return

    _orig_compile = bass_utils.compile_bir_kernel

    def _split_multi_waits(bir_json):
        m = _json.loads(bir_json)
        counter = 0
        changed = False
        for fn in m["functions"]:
            for blk in fn["blocks"]:
                new_insts = []
                for inst in blk["instructions"]:
                    si = inst.get("sync_info")
                    waits = (si or {}).get("on_wait") or []
                    sem_waits = [w for w in waits if w.get("sync_type") == "semaphore"]
                    if len(waits) > 1 and len(sem_waits) == len(waits):
                        changed = True
                        for w in waits[:-1]:
                            counter += 1
                            new_insts.append({
                                "name": f"I-wsplit{counter}",
                                "opcode": "NoOp",
                                "engine": inst["engine"],
                                "ins": [],
                                "outs": [],
                                "sync_info": {"on_wait": [w], "on_update": []},
                            })
                        si["on_wait"] = [waits[-1]]
                    new_insts.append(inst)
                blk["instructions"] = new_insts
        if not changed:
            return bir_json
        return _json.dumps(m).encode()

    def _compile_bir_kernel_split(bir_json, tmpdir, neff_name="file.neff"):
        return _orig_compile(_split_multi_waits(bir_json), tmpdir, neff_name=neff_name)

    _compile_bir_kernel_split._wait_split = True
    bass_utils.compile_bir_kernel = _compile_bir_kernel_split
    bass2jax.compile_bir_kernel = _compile_bir_kernel_split


def build_kernel():
    _patch_tile_drain()
    nc = bass.Bass(target_bir_lowering=False, trn_type="TRN2")

    # hi/lo fp8 operand pairs; layouts are DoubleRow-packed on the host:
    # contraction index c = ko*256 + slot*128 + p.
    xh = nc.dram_tensor("xh", [NCHUNK, 128, 2, KO2, CHUNK], F8, kind="ExternalInput")
    xl = nc.dram_tensor("xl", [NCHUNK, 128, 2, KO2, CHUNK], F8, kind="ExternalInput")
    wqh = nc.dram_tensor("wqh", [128, 2, KO2, DG], F8, kind="ExternalInput")
    wql = nc.dram_tensor("wql", [128, 2, KO2, DG], F8, kind="ExternalInput")
    wkh = nc.dram_tensor("wkh", [128, 2, KO2, DG], F8, kind="ExternalInput")
    wkl = nc.dram_tensor("wkl", [128, 2, KO2, DG], F8, kind="ExternalInput")
    wvh = nc.dram_tensor("wvh", [128, 2, KO2, DG], F8, kind="ExternalInput")
    wvl = nc.dram_tensor("wvl", [128, 2, KO2, DG], F8, kind="ExternalInput")
    wpt = nc.dram_tensor("wpt", [128, 2, C], BF16, kind="ExternalInput")
    bq4 = nc.dram_tensor("bq4", [128, 2], F32, kind="ExternalInput")
    bk4 = nc.dram_tensor("bk4", [128, 2], F32, kind="ExternalInput")
    out = nc.dram_tensor("out", [NKT, 128, C], BF16, kind="ExternalOutput")

    from contextlib import ExitStack

    with tile.TileContext(nc) as tc, ExitStack() as ctx:
        from concourse.masks import make_identity

        const = ctx.enter_context(tc.tile_pool(name="const", bufs=1))
        xpool = ctx.enter_context(tc.tile_pool(name="xp", bufs=4))
        persist = ctx.enter_context(tc.tile_pool(name="persist", bufs=1))
        epool = ctx.enter_context(tc.tile_pool(name="ep", bufs=5))
        ypool = ctx.enter_context(tc.tile_pool(name="yp", bufs=2))
        ytpool = ctx.enter_context(tc.tile_pool(name="ytp", bufs=4))
        opool = ctx.enter_context(tc.tile_pool(name="op", bufs=3))
        small = ctx.enter_context(tc.tile_pool(name="sm", bufs=6))
        ps_big = ctx.enter_context(tc.tile_pool(name="psb", bufs=2, space="PSUM"))
        ps_y = ctx.enter_context(tc.tile_pool(name="psy", bufs=2, space="PSUM"))
        ps_o = ctx.enter_context(tc.tile_pool(name="pso", bufs=2, space="PSUM"))

        _x_tiles = {}

        def prefetch_x(n):
            if n not in _x_tiles and n < NCHUNK:
                th = xpool.tile([128, 2, KO2, CHUNK], F8, tag="x", name=f"xh{n}")
                nc.sync.dma_start(th[:], xh[n])
                tl = xpool.tile([128, 2, KO2, CHUNK], F8, tag="x", name=f"xl{n}")
                nc.sync.dma_start(tl[:], xl[n])
                _x_tiles[n] = (th, tl)

        def load_x(n):
            prefetch_x(n)
            return _x_tiles.pop(n)

        # ---- constants ----  (x chunk 0 is prefetched right after the wq
        # pair so the first projection matmul can start ~2.5us in)
        wq_sb, wk_sb, wv_sb = [], [], []
        _w_srcs = ((wq_sb, wqh, wql), (wk_sb, wkh, wkl), (wv_sb, wvh, wvl))
        _w_tiles = []
        for wn, (dst, hi, lo) in enumerate(_w_srcs):
            for hl, w_dram in enumerate((hi, lo)):
                t = const.tile([128, 2, KO2, DG], F8, name=f"w{wn}_{hl}")
                _w_tiles.append((t, w_dram))
                dst.append(t)
        _order = [0, 2, 1, 3, 4, 5]        # wq-hi, wk-hi, wq-lo, wk-lo, wv
        nc.sync.dma_start(_w_tiles[0][0][:], _w_tiles[0][1][:])   # wq hi
        nc.sync.dma_start(_w_tiles[2][0][:], _w_tiles[2][1][:])   # wk hi
        prefetch_x(0)
        for wi in (1, 3, 4, 5):
            t, w_dram = _w_tiles[wi]
            nc.sync.dma_start(t[:], w_dram[:])
        bq_sb = const.tile([128, 2], F32)
        nc.sync.dma_start(bq_sb[:], bq4[:])
        bk_sb = const.tile([128, 2], F32)
        nc.sync.dma_start(bk_sb[:], bk4[:])
        wpt_sb = const.tile([128, 2, C], BF16)
        nc.sync.dma_start(wpt_sb[:], wpt[:])

        ident = const.tile([128, 128], BF16)
        make_identity(nc, ident)
        ones_row = const.tile([1, 128], BF16)
        nc.vector.memset(ones_row[:], 1.0)
        zero_row = const.tile([1, 512], BF16)
        nc.vector.memset(zero_row[:], 0.0)
        # wmask[k, q] = 0 where q >= k else MASK_NEG (strict lower triangle
        # of keys over queries within the diagonal 128x128 block)
        wmask = const.tile([128, 128], BF16)
        nc.gpsimd.memset(wmask[:], 0.0)
        nc.gpsimd.affine_select(
            out=wmask[:],
            in_=wmask[:],
            compare_op=mybir.AluOpType.is_ge,
            fill=MASK_NEG,
            base=0,
            pattern=[[1, 128]],
            channel_multiplier=-1,
        )

        # ---- persistent activations ----
        # q8/k8: partition = (h%2)*64 + dh, free dims (pair, drslot, t).
        # drslot 1 is a constant zero operand: DoubleRow needs a [p, 2, n]
        # shape but the contraction is only 64 deep, so the second slot
        # multiplies zeros (and must be zeroed -- fp8 garbage can be NaN).
        q8 = persist.tile([128, 2, 2, T], F8)
        k8 = persist.tile([128, 2, 2, T], F8)
        nc.gpsimd.memset(q8[:, :, 1, :], 0.0)
        nc.gpsimd.memset(k8[:, :, 1, :], 0.0)
        # v: partition = key%128, free (ktile, head, dh+ones)
        v_sb = persist.tile([128, NKT, HPG, DH + 1], BF16)
        nc.vector.memset(v_sb[:, :, :, DH:DH + 1], 1.0)

        def proj_units(n):
            """Chunk-n projection emission as self-contained closures (one
            complete PSUM accumulation group each) so they can be spread
            across the previous chunk's exp-bound attention phase."""
            if n >= NCHUNK:
                return []
            cols = slice(n * CHUNK, (n + 1) * CHUNK)
            xs = {}

            def get_x():
                if "x" not in xs:
                    xs["x"] = load_x(n)
                    prefetch_x(n + 1)
                return xs["x"]

            units = []

            def qk_unit(w_pair, b_sb, dst, mt, tag):
                def run():
                    xthi, xtlo = get_x()
                    terms = ((xthi, 0), (xthi, 1), (xtlo, 0))
                    ps = ps_big.tile([128, 2, CHUNK], F32, tag="ps",
                                     name=f"p{tag}{n}_{mt}")
                    i, nmm = 0, len(terms) * KO2
                    for xt, wi in terms:
                        for ko in range(KO2):
                            nc.tensor.matmul(
                                ps[:, 0, :],
                                lhsT=w_pair[wi][:, :, ko, mt * 128:(mt + 1) * 128],
                                rhs=xt[:, :, ko, :],
                                start=(i == 0),
                                stop=(i == nmm - 1),
                                perf_mode=DR,
                            )
                            i += 1
                    nc.vector.tensor_scalar(
                        dst[:, mt, 0, cols], ps[:, 0, :],
                        SQ / (SX * SW), b_sb[:, mt:mt + 1],
                        op0=mybir.AluOpType.mult, op1=mybir.AluOpType.add,
                    )
                return run

            def v_unit(tt):
                def run():
                    xthi, xtlo = get_x()
                    terms = ((xthi, 0), (xthi, 1), (xtlo, 0))
                    kt = 4 * n + tt
                    ps = ps_big.tile([128, 2, CHUNK], F32, tag="ps",
                                     name=f"pv{n}_{tt}")
                    i, nmm = 0, len(terms) * KO2
                    for xt, wi in terms:
                        for ko in range(KO2):
                            nc.tensor.matmul(
                                ps[:, 0, 0:DG],
                                lhsT=xt[:, :, ko, tt * 128:(tt + 1) * 128],
                                rhs=wv_sb[wi][:, :, ko, :],
                                start=(i == 0),
                                stop=(i == nmm - 1),
                                perf_mode=DR,
                            )
                            i += 1
                    nc.vector.tensor_scalar_mul(
                        v_sb[:, kt, :, 0:DH], ps[:, 0, 0:DG], 1.0 / (SX * SW)
                    )
                return run

            for mt in range(2):
                units.append(qk_unit(wq_sb, bq_sb, q8, mt, "q"))
                units.append(qk_unit(wk_sb, bk_sb, k8, mt, "k"))
            for tt in range(4):
                units.append(v_unit(tt))
            return units

        DELAY = 3

        carry_out = []
        pre_scores = {}

        def emit_scores(sn, p, j):
            """Score matmuls + causal mask + exp for (chunk sn, pair p,
            key tile j); returns the bf16 e tile."""
            qlo = max(0, 128 * (j - 4 * sn))
            diag = j >= 4 * sn
            pss = ps_big.tile([128, 2, CHUNK], F32, tag="ps",
                              name=f"ss{sn}_{p}_{j}")
            for h01 in range(2):
                rows = slice(64 * h01, 64 * h01 + 64)
                nc.tensor.matmul(
                    pss[:, h01, qlo:],
                    lhsT=k8[rows, p, :, j * 128:(j + 1) * 128],
                    rhs=q8[rows, p, :,
                           sn * CHUNK + qlo:(sn + 1) * CHUNK],
                    start=True,
                    stop=not diag,
                    perf_mode=DR,
                )
            if diag:
                for h01 in range(2):
                    nc.tensor.matmul(
                        pss[:, h01, qlo:qlo + 128],
                        lhsT=ident[:],
                        rhs=wmask[:],
                        start=False,
                        stop=True,
                    )
            e = epool.tile([128, 2, CHUNK], BF16, tag="e")
            nc.scalar.activation(
                e[:, :, qlo:], pss[:, :, qlo:],
                mybir.ActivationFunctionType.Exp,
                scale=1.0 / (8.0 * SQ * SQ),
            )
            return e

        def attention(n, feed):
            """Scores + attn@v for chunk n (two head-pair passes). attn@v
            runs DELAY score-tiles behind the exp producing its input, and
            units from `feed` (next chunk's projection) are spread evenly
            over the score tiles to fill the PE while ACT works through
            the exps."""
            njt = 4 * (n + 1)
            steps_total = 2 * njt
            step_no = [0]
            nfeed = len(feed)
            consumed = [0]
            carried = list(carry_out)
            carry_out.clear()
            y_sb = ypool.tile([128, 4, HPG, DH], BF16, tag="y", name=f"y{n}")
            yt_sb = ytpool.tile([128, 2, CHUNK], BF16, tag="yt", name=f"yt{n}")
            pending_av = []
            pending_out = []

            def normalize(p, qt, psyA, psyB):
                rec = small.tile([128, 2], F32, tag="rec", name=f"rc{n}_{p}_{qt}")
                lv = small.tile([128, 2], F32, tag="lv", name=f"lv{n}_{p}_{qt}")
                nc.vector.tensor_copy(lv[:, 0:1], psyA[:, qt, DH:DH + 1])
                nc.vector.tensor_copy(lv[:, 1:2], psyB[:, qt, DH:DH + 1])
                nc.vector.reciprocal(rec[:], lv[:])
                for h01, psy in ((0, psyA), (1, psyB)):
                    nc.vector.tensor_scalar_mul(
                        y_sb[:, qt, 2 * p + h01, :], psy[:, qt, 0:DH],
                        rec[:, h01:h01 + 1],
                    )

            def transpose_y(qt):
                # y[q, ydim] -> yT[ydim, t] via the PE transpose datapath;
                # output lands bf16 in a bitcast corner of an outproj-pool
                # tile (keeps the scores-pool rotation free of chunk-tail
                # eviction dependencies)
                tp = ps_big.tile([128, 2, CHUNK], F32, tag="ps", name=f"tp{n}_{qt}")
                tpb = tp[:].bitcast(BF16)
                for mt in range(2):
                    nc.tensor.matmul(
                        tpb[:, mt, 0:128],
                        lhsT=y_sb[:, qt, 2 * mt:2 * mt + 2, :],
                        rhs=ident[:],
                        is_transpose=True,
                    )
                nc.vector.tensor_copy(
                    yt_sb[:, :, qt * 128:(qt + 1) * 128], tpb[:, :, 0:128]
                )

            def outproj(qt):
                t_tile = 4 * n + qt
                o_sb = opool.tile([128, C], BF16, tag="o", name=f"o{n}_{qt}")
                for nh in range(2):
                    ps = ps_o.tile([128, 512], F32, tag="o", name=f"po{n}_{qt}_{nh}")
                    for mt in range(2):
                        nc.tensor.matmul(
                            ps[:],
                            lhsT=yt_sb[:, mt, qt * 128:(qt + 1) * 128],
                            rhs=wpt_sb[:, mt, nh * 512:(nh + 1) * 512],
                            start=(mt == 0),
                            stop=(mt == 1),
                        )
                    nc.vector.tensor_copy(o_sb[:, nh * 512:(nh + 1) * 512], ps[:])
                nc.sync.dma_start(out[t_tile], o_sb[:])

            def attnv(p, j, e, psyA, psyB):
                def run():
                    qlo = max(0, 128 * (j - 4 * n))
                    for qt in range(qlo // 128, 4):
                        for h01, psy in ((0, psyA), (1, psyB)):
                            nc.tensor.matmul(
                                psy[:, qt, 0:DH + 1],
                                lhsT=e[:, h01, qt * 128:(qt + 1) * 128],
                                rhs=v_sb[:, j, 2 * p + h01, :],
                                start=False,
                                stop=(j == 4 * n + qt),
                                skip_group_check=True,
                            )
                    if j >= 4 * n:
                        qt_done = j - 4 * n
                        normalize(p, qt_done, psyA, psyB)
                        if p == 1:
                            transpose_y(qt_done)
                            if n < NCHUNK - 1:
                                carry_out.append(
                                    lambda qt=qt_done: outproj(qt))
                            else:
                                pending_out.append(qt_done)
                                if len(pending_out) > 1:
                                    outproj(pending_out.pop(0))
                return run

            def step():
                if pending_av and len(pending_av) > DELAY:
                    pending_av.pop(0)()
                step_no[0] += 1
                if carried and step_no[0] % 2 == 0:
                    carried.pop(0)()
                # spread feed units so they finish ~7/8 through the chunk
                if feed:
                    target = min(len(feed) + consumed[0],
                                 (nfeed * 8 * step_no[0])
                                 // (7 * steps_total) + 1)
                    while consumed[0] < target and feed:
                        feed.pop(0)()
                        consumed[0] += 1

            for p in range(2):
                psyA = ps_y.tile([128, 4, 128], F32, tag="psy", name=f"pyA{n}_{p}")
                psyB = ps_y.tile([128, 4, 128], F32, tag="psy", name=f"pyB{n}_{p}")
                # one start per PSUM bank: a K=1 matmul zeroes the whole
                # bank so every attn@v slot can accumulate with start=False
                # (multiple start groups in one 2KB region clobber siblings)
                for psy in (psyA, psyB):
                    nc.tensor.matmul(
                        psy[:, :, :].rearrange("p a b -> p (a b)"),
                        lhsT=ones_row[:],
                        rhs=zero_row[:],
                        start=True,
                        stop=True,
                        skip_group_check=True,
                    )
                for j in range(njt):
                    e = pre_scores.pop((n, p, j), None)
                    if e is None:
                        e = emit_scores(n, p, j)
                    if p == 1 and j == njt - 1 and n + 1 < NCHUNK:
                        # pre-emit the next chunk's first two score tiles so
                        # ACT streams straight through the chunk boundary
                        for jj in range(2):
                            pre_scores[(n + 1, 0, jj)] = emit_scores(
                                n + 1, 0, jj)
                    pending_av.append(attnv(p, j, e, psyA, psyB))
                    step()
            while pending_av:
                pending_av.pop(0)()
                while feed:
                    feed.pop(0)()
            for fn in carried:
                fn()
            if n < NCHUNK - 1:
                for qt in pending_out:
                    carry_out.append(lambda qt=qt: outproj(qt))
            else:
                for qt in pending_out:
                    outproj(qt)

        for u in proj_units(0):
            u()
        for n in range(NCHUNK):
            feed = proj_units(n + 1)
            attention(n, feed)
            for u in feed:
                u()

    return nc


_NC_CACHE = None


def _split8(a, s):
    hi = np.asarray(a * s, NPF8)
    lo = np.asarray(a * s - hi.astype(np.float32), NPF8)
    return hi, lo


def kernel(**inputs) -> np.ndarray:
    global _NC_CACHE
    x = np.asarray(inputs["x"], np.float32)
    Wq = np.asarray(inputs["Wq"], np.float32)
    Wk = np.asarray(inputs["Wk"], np.float32)
    Wv = np.asarray(inputs["Wv"], np.float32)
    Wp = np.asarray(inputs["Wp"], np.float32)
    bq = np.asarray(inputs["bq"], np.float32)
    bk = np.asarray(inputs["bk"], np.float32)
    bv = np.asarray(inputs["bv"], np.float32)
    bp = np.asarray(inputs["bp"], np.float32)

    if _NC_CACHE is None:
        _NC_CACHE = build_kernel()
    nc = _NC_CACHE

    def pack_w(Wl):
        # Wl: [256 out, 1024 in] slice -> lhsT [c, m] -> [p, slot, ko, m]
        wt = Wl.T                                          # [1024 c, 256 m]
        wt = wt.reshape(KO2, 2, 128, 256).transpose(2, 1, 0, 3)
        hi, lo = _split8(np.ascontiguousarray(wt), SW)
        return np.ascontiguousarray(hi), np.ascontiguousarray(lo)

    in_maps = []
    for c in range(NCORES):
        b, g = divmod(c, GROUPS)
        rows = slice(g * DG, (g + 1) * DG)
        xt = x[b].T.reshape(KO2, 2, 128, T).transpose(2, 1, 0, 3)  # [p,slot,ko,t]
        xt = xt.reshape(128, 2, KO2, NCHUNK, CHUNK).transpose(3, 0, 1, 2, 4)
        xhi, xlo = _split8(np.ascontiguousarray(xt), SX)

        wq_hi, wq_lo = pack_w(Wq[rows])
        wk_hi, wk_lo = pack_w(Wk[rows])
        wv_hi, wv_lo = pack_w(Wv[rows])
        wpt_l = np.ascontiguousarray(
            Wp[:, rows].T.reshape(2, 128, C).transpose(1, 0, 2)
        ).astype(NPBF16)

        bq4 = np.ascontiguousarray((bq[rows] * SQ).reshape(2, 128).T)
        bk4 = np.ascontiguousarray((bk[rows] * SQ).reshape(2, 128).T)

        in_maps.append({
            "xh": np.ascontiguousarray(xhi),
            "xl": np.ascontiguousarray(xlo),
            "wqh": wq_hi, "wql": wq_lo,
            "wkh": wk_hi, "wkl": wk_lo,
            "wvh": wv_hi, "wvl": wv_lo,
            "wpt": wpt_l,
            "bq4": bq4, "bk4": bk4,
        })

    res = run_bass_kernel_spmd(nc, in_maps, core_ids=list(range(NCORES)))

    result = np.zeros((B, T, C), np.float32)
    for c in range(NCORES):
        b = c // GROUPS
        o = np.asarray(res.results[c]["out"]).astype(np.float32)
        result[b] += o.reshape(T, C)
    result += (bv @ Wp.T + bp)[None, None, :]
    return result
